# revision 10
# baseline (speedup 1.0000x reference)
"""Trainium2 Bass kernel for MessagePassingWithPhase (v2).

Reference computation (B=2, N=512, D=128, O=4):
    recv = X @ W1r ; send = X @ W1s
    hidden[b,i,j,:]  = relu(recv[b,i] + send[b,j] + b1)
    messages         = hidden @ W2 + b2
    gate             = sigmoid(cos(phi_i - phi_j) @ Wg + bg)
    agg[b,i]         = sum_j mask[i,j] * (messages * gate)[b,i,j] / cnt_i
    out              = X + (relu(X@Wu1x + agg@Wu1a + bu1) @ Wu2 + bu2)

Mapping (8 cores, receiver axis sharded: 64 receivers/core, both batches),
feature-major layout (D=128 partitions, node index on the free axis).

Execution-environment model (measured on this axon/PJRT path):
  * Static instructions cost ~30-70us each PER EXECUTE, but re-running them
    in a hardware loop is ~free -> REPEAT is a tc.For_i hardware loop, so
    the repeat-delta used for timing measures true device time.
  * Inside the loop: DMA instructions still cost ~60us each -> no DMA in
    the loop body (masks are staged to SBUF once; output DMA'd after).
  * DVE ops on <128-partition tiles and PSUM-source DVE ops are 5-20x
    slower than modeled; bf16 DVE ops are ~7x slower than fp32 -> all DVE
    work is fp32/f32r on full 128-partition tiles, SBUF-to-SBUF.
  * ACT (scalar engine) reads PSUM at full speed -> all PSUM evacuation
    (sigmoid, +bias copies) happens on ACT.
  * GPSIMD elementwise is free-running in-loop -> gate multiply + fold
    tree run there, in parallel with DVE.

Gate path trick: cos(phi_i-phi_j) = cos_i cos_j + sin_i sin_j, so with
P8 = [cos(phi); sin(phi)] (8, N) and Wg8 = [Wg; Wg]:
    glin[d, i, j] = sum_o (Wg8[o,d] * P8r[o,i]) * P8[o,j] + mneg[i,j]
The per-receiver stationary Wi[o,d] = Wg8[o,d]*P8r[o,i] is built for 16
receivers at a time directly in packed-partition orientation (partition
32g+o holds receiver i=4c+g, chunk c on the free axis) by ONE full-width
DVE multiply - no transposes, no partial-partition ops.  The mask enters
as a K=1 fp8 matmul accumulating -48*(1-mask) into the gate PSUM, so the
sigmoid saturates to ~0 on non-neighbors and masked-mean becomes a plain
sum scaled by 1/cnt_i.  bg is the sigmoid's bias AP; b2 is added by the
ACT evacuation of the message PSUM.
"""
import os
import sys
import numpy as np

for _p in ("/opt/trn_rl_repo", "/root/.axon_site/_ro/trn_rl_repo"):
    if os.path.isdir(_p) and _p not in sys.path:
        sys.path.append(_p)

B, N, D, O = 2, 512, 128, 4
NCORES = 8
NPC = N // NCORES       # receivers per core
MASK_NEG = -48.0        # exact in fp8e4m3; sigmoid(-48+eps) ~ 1e-20
QUAD = 16               # receivers per block (pre/H/M/G tile granularity)
NBLK = NPC // QUAD      # iq blocks per batch
NCH = NPC // 4          # receiver chunks (4 receivers each) per batch

REPEAT = 1              # timing aid: loop trip count of the device For_i

# engine knobs for the multiply + fold-tree ("vector" or "gpsimd")
MG_ENGINE = "gpsimd"
FOLD_ENGINE = "gpsimd"

# timing-ablation: set of stage names to OMIT from the loop body (timing
# experiments only -- output is wrong when non-empty)
ABLATE = frozenset()

_CACHE = {}


def _build_program():
    import concourse.bacc as bacc
    import concourse.mybir as mybir
    import concourse.tile as tile

    f32 = mybir.dt.float32
    f32r = mybir.dt.float32r
    f8 = mybir.dt.float8e4
    A = mybir.AluOpType
    AF = mybir.ActivationFunctionType

    nc = bacc.Bacc("TRN2", debug=False)

    def din(name, shape, dt=f32):
        return nc.declare_dram_parameter(name, list(shape), dt, isOutput=False)

    xt = din("xt", (B, D, N))            # node features, transposed
    xtr = din("xtr", (D, B * NPC))       # receiver cols of xt, both batches
    p8q = din("p8q", (B, D, N))          # P8 replicated at partitions 32g+o
    p8rc = din("p8rc", (B, D, NCH))      # P8r packed: [32g+o, c] = P8r[o, 4c+g]
    # [w1r|w1s|w2|wu1x|wu1a|wu2|wg8stack|cinv(B*NPC)|b1|bg|b2|bu1|bu2]
    NBLOB = 7 * D + B * NPC + 5
    blob = din("blob", (D, NBLOB))
    mneg8 = din("mneg8", (1, NPC * N), f8)   # -48*(1-mask), receiver-major
    ones1 = din("ones1", (1, D), f8)
    out = nc.declare_dram_parameter("out", [B, D, NPC], f32, isOutput=True)

    with tile.TileContext(nc) as tc:
        with (
            tc.tile_pool(name="const", bufs=1) as cp,
            tc.tile_pool(name="work", bufs=1) as wp,
            tc.tile_pool(name="psA", bufs=1, space="PSUM") as psA,
        ):
            def ct(dram, shape, dt=f32, tag=None):
                t = cp.tile(list(shape), dt, tag=tag, name=tag)
                nc.sync.dma_start(t[:], dram[:])
                return t

            blob_t = ct(blob, (D, NBLOB), tag="blob")
            w1r_t = blob_t[:, 0 * D:1 * D]
            w1s_t = blob_t[:, 1 * D:2 * D]
            w2_f = blob_t[:, 2 * D:3 * D]
            wu1x_t = blob_t[:, 3 * D:4 * D]
            wu1a_t = blob_t[:, 4 * D:5 * D]
            wu2_t = blob_t[:, 5 * D:6 * D]
            wg8s_t = blob_t[:, 6 * D:7 * D]
            cinv_t = blob_t[:, 7 * D:7 * D + B * NPC]
            bofs = 7 * D + B * NPC
            b1c = blob_t[:, bofs + 0:bofs + 1]
            bgc = blob_t[:, bofs + 1:bofs + 2]
            b2c = blob_t[:, bofs + 2:bofs + 3]
            bu1c = blob_t[:, bofs + 3:bofs + 4]
            bu2c = blob_t[:, bofs + 4:bofs + 5]

            # fp32r copies must be produced on-device (rounded-producer rule)
            w2r = cp.tile([D, D], f32r, tag="w2r", name="w2r")
            nc.vector.tensor_copy(w2r[:], w2_f)

            xt_t, p8qr_t, p8rc_t = [], [], []
            for b in range(B):
                xt_b = ct(xt[b], (D, N), tag=f"xt{b}")
                p8q_b = ct(p8q[b], (D, N), tag=f"p8q{b}")
                p8qr_b = cp.tile([D, N], f32r, tag=f"p8qr{b}", name=f"p8qr{b}")
                nc.vector.tensor_copy(p8qr_b[:], p8q_b[:])
                p8rc_b = ct(p8rc[b], (D, NCH), tag=f"p8rc{b}")
                xt_t.append(xt_b)
                p8qr_t.append(p8qr_b)
                p8rc_t.append(p8rc_b)
            xtr_all = ct(xtr, (D, B * NPC), tag="xtr")
            mneg_t = ct(mneg8, (1, NPC * N), f8, tag="mneg")
            ones_t = ct(ones1, (1, D), f8, tag="ones")

            araw = cp.tile([D, B * NPC], f32, tag="araw", name="araw")
            sendT = [cp.tile([D, N], f32, tag=f"send{b}", name=f"send{b}")
                     for b in range(B)]
            recvb = cp.tile([D, B * NPC], f32, tag="recvb", name="recvb")
            wi_pack = [cp.tile([D, NCH * D], f32r, tag=f"wi{b}", name=f"wi{b}")
                       for b in range(B)]
            o_all = cp.tile([D, B * NPC], f32, tag="o_all", name="o_all")

            eng = {"vector": nc.vector, "gpsimd": nc.gpsimd}
            mg_e = eng[MG_ENGINE]
            fold_e = eng[FOLD_ENGINE]

            with tc.For_i(0, REPEAT, 1):
              ab = ABLATE
              if True:
                # ---- projections (send for all j, recv for local receivers)
                for b in range(B if "proj" not in ab else 0):
                    s_ps = psA.tile([D, N], f32, tag="mq", name="s_ps")
                    nc.tensor.matmul(s_ps[:], w1s_t, xt_t[b][:],
                                     start=True, stop=True)
                    nc.scalar.copy(sendT[b][:], s_ps[:])
                if "proj" not in ab:
                    r_ps = psA.tile([D, B * NPC], f32, tag="gq", name="r_ps")
                    nc.tensor.matmul(r_ps[:], w1r_t, xtr_all[:],
                                     start=True, stop=True)
                    nc.scalar.add(recvb[:], r_ps[:], b1c)

                # ---- per-receiver gate stationaries, packed orientation
                for b in range(B if "wi" not in ab else 0):
                    nc.vector.tensor_tensor(
                        wi_pack[b][:].rearrange("p (a b) -> p a b", a=NCH),
                        wg8s_t.unsqueeze(1).broadcast_to((D, NCH, D)),
                        p8rc_t[b][:].unsqueeze(2).broadcast_to((D, NCH, D)),
                        A.mult)

                for b in range(B):
                    for iq in range(NBLK):
                        i0 = iq * QUAD
                        # hidden pre-activation for QUAD receivers (DVE)
                        pre = wp.tile([D, QUAD * N], f32, tag="pre", name="pre")
                        if "pre" not in ab:
                          nc.vector.tensor_tensor(
                            pre[:].rearrange("p (a b) -> p a b", a=QUAD),
                            sendT[b][:].unsqueeze(1)
                                .broadcast_to((D, QUAD, N)),
                            recvb[:, b * NPC + i0:b * NPC + i0 + QUAD]
                                .unsqueeze(2).broadcast_to((D, QUAD, N)),
                            A.add)
                        # relu on ACT, producing the f32r matmul operand
                        H = wp.tile([D, QUAD * N], f32r, tag="H", name="H")
                        if "relu" not in ab:
                            nc.scalar.activation(H[:], pre[:], AF.Relu)

                        Msb = wp.tile([D, QUAD * N], f32, tag="Msb", name="Msb")
                        Gsb = wp.tile([D, QUAD * N], f32, tag="Gsb", name="Gsb")
                        for cl in range(QUAD // 4):
                            c = iq * (QUAD // 4) + cl    # global chunk
                            mq = psA.tile([D, 4 * N], f32, tag="mq", name="mq")
                            gq = psA.tile([D, 4 * N], f32, tag="gq", name="gq")
                            for k in range(4 if "msg" not in ab else 0):
                                nc.tensor.matmul(
                                    mq[:, k * N:(k + 1) * N], w2r[:],
                                    H[:, (cl * 4 + k) * N:(cl * 4 + k + 1) * N],
                                    start=True, stop=True)
                            for k in range(4 if "gate" not in ab else 0):
                                i_g = c * 4 + k
                                nc.tensor.matmul(
                                    gq[:, k * N:(k + 1) * N],
                                    wi_pack[b][32 * k:32 * k + 2 * O,
                                               c * D:(c + 1) * D],
                                    p8qr_t[b][32 * k:32 * k + 2 * O, :],
                                    start=True, stop=False,
                                    tile_position=(32 * k, 0))
                                nc.tensor.matmul(
                                    gq[:, k * N:(k + 1) * N], ones_t[:],
                                    mneg_t[0:1, i_g * N:(i_g + 1) * N],
                                    start=False, stop=True)
                            # PSUM evacuation on ACT: sigmoid(+bg), msg+b2
                            if "evac" not in ab:
                                nc.scalar.activation(
                                    Gsb[:, cl * 4 * N:(cl + 1) * 4 * N], gq[:],
                                    AF.Sigmoid, bias=bgc)
                                nc.scalar.add(
                                    Msb[:, cl * 4 * N:(cl + 1) * 4 * N], mq[:], b2c)
                        # gated messages (pre is dead; reuse as the product)
                        if "mg" not in ab:
                            mg_e.tensor_tensor(pre[:], Msb[:], Gsb[:], A.mult)
                        # fold tree over j: 512 -> 32, ping-pong pre <-> Gsb
                        src, dst = pre, Gsb
                        w = N // 2
                        while w > QUAD and "folds" not in ab:
                            fold_e.tensor_tensor(
                                dst[:, :QUAD * w]
                                    .rearrange("p (a b) -> p a b", a=QUAD),
                                src[:, :QUAD * 2 * w]
                                    .rearrange("p (a b) -> p a b", a=QUAD)[:, :, 0:w],
                                src[:, :QUAD * 2 * w]
                                    .rearrange("p (a b) -> p a b", a=QUAD)[:, :, w:2 * w],
                                A.add)
                            src, dst = dst, src
                            w //= 2
                        # final per-receiver sums (DVE, small)
                        if "red" not in ab:
                          nc.vector.reduce_sum(
                            araw[:, b * NPC + i0:b * NPC + i0 + QUAD],
                            src[:, :QUAD * 2 * w]
                                .rearrange("p (a b) -> p a b", a=QUAD),
                            axis=mybir.AxisListType.X)

                # ---- masked-mean scale + update MLP + residual
                if "tail" in ab:
                    nc.vector.tensor_copy(o_all[:], xtr_all[:])
                else:
                    aggt = wp.tile([D, B * NPC], f32, tag="aggt", name="aggt")
                    nc.vector.tensor_tensor(aggt[:], araw[:], cinv_t, A.mult)
                    u_ps = psA.tile([D, B * NPC], f32, tag="mq", name="u_ps")
                    nc.tensor.matmul(u_ps[:], wu1x_t, xtr_all[:],
                                     start=True, stop=False)
                    nc.tensor.matmul(u_ps[:], wu1a_t, aggt[:],
                                     start=False, stop=True)
                    hT = wp.tile([D, B * NPC], f32, tag="hT", name="hT")
                    nc.scalar.activation(hT[:], u_ps[:], AF.Relu, bias=bu1c)
                    o_ps = psA.tile([D, B * NPC], f32, tag="gq", name="o_ps")
                    nc.tensor.matmul(o_ps[:], wu2_t, hT[:], start=True, stop=True)
                    o1 = wp.tile([D, B * NPC], f32, tag="o1", name="o1")
                    nc.scalar.add(o1[:], o_ps[:], bu2c)
                    nc.vector.tensor_tensor(o_all[:], o1[:], xtr_all[:], A.add)

            nc.sync.dma_start(out[:].rearrange("b d n -> d b n"),
                              o_all[:].rearrange("d (b n) -> d b n", b=B))

    nc.compile()
    return nc


def _get_program():
    key = (REPEAT, QUAD, MG_ENGINE, FOLD_ENGINE, ABLATE)
    if key not in _CACHE:
        _CACHE[key] = _build_program()
    return _CACHE[key]


_RUNNERS = {}


def _get_runner(nc):
    """Cached PJRT runner for a compiled program.

    ``bass_utils.run_bass_kernel_spmd`` builds a fresh ``shard_map`` +
    ``jax.jit`` closure on every call, so every kernel() call pays a full
    retrace/lower (~100-300ms, noisy).  Building the jitted executable once
    per program makes per-call wall time small and stable, which the
    repeat-delta timing method depends on.
    """
    if id(nc) in _RUNNERS:
        return _RUNNERS[id(nc)]

    import jax
    import concourse.mybir as mybir
    from concourse import bass2jax
    from jax.sharding import Mesh, PartitionSpec
    from jax.experimental.shard_map import shard_map

    bass2jax.install_neuronx_cc_hook()

    partition_name = (nc.partition_id_tensor.name
                      if nc.partition_id_tensor else None)
    in_names, out_names, out_avals, zero_shapes = [], [], [], []
    for alloc in nc.m.functions[0].allocations:
        if not isinstance(alloc, mybir.MemoryLocationSet):
            continue
        name = alloc.memorylocations[0].name
        if alloc.kind == "ExternalInput":
            if name != partition_name:
                in_names.append(name)
        elif alloc.kind == "ExternalOutput":
            shape = tuple(alloc.tensor_shape)
            dtype = mybir.dt.np(alloc.dtype)
            out_names.append(name)
            out_avals.append(jax.core.ShapedArray(shape, dtype))
            zero_shapes.append((shape, dtype))
    n_params = len(in_names)
    n_outs = len(out_avals)
    all_names = list(in_names) + list(out_names)
    if partition_name is not None:
        all_names.append(partition_name)
    donate = tuple(range(n_params, n_params + n_outs))

    def _body(*args):
        operands = list(args)
        if partition_name is not None:
            operands.append(bass2jax.partition_id_tensor())
        outs = bass2jax._bass_exec_p.bind(
            *operands,
            out_avals=tuple(out_avals),
            in_names=tuple(all_names),
            out_names=tuple(out_names),
            lowering_input_output_aliases=(),
            sim_require_finite=True,
            sim_require_nnan=True,
            nc=nc,
        )
        return tuple(outs)

    devices = jax.devices()[:NCORES]
    mesh = Mesh(np.asarray(devices), ("core",))
    in_specs = (PartitionSpec("core"),) * (n_params + n_outs)
    out_specs = (PartitionSpec("core"),) * n_outs
    sharded = jax.jit(
        shard_map(_body, mesh=mesh, in_specs=in_specs, out_specs=out_specs,
                  check_rep=False),
        donate_argnums=donate, keep_unused=True,
    )

    def run(in_maps):
        concat_in = [
            np.concatenate([np.asarray(m[name]) for m in in_maps], axis=0)
            for name in in_names
        ]
        concat_zeros = [
            np.zeros((NCORES * s[0], *s[1:]), dt) for s, dt in zero_shapes
        ]
        out_arrs = sharded(*concat_in, *concat_zeros)
        return [
            {
                name: np.asarray(out_arrs[i]).reshape(
                    NCORES, *zero_shapes[i][0])[c]
                for i, name in enumerate(out_names)
            }
            for c in range(NCORES)
        ]

    _RUNNERS[id(nc)] = run
    return run


def kernel(node_features, node_phases, adjacency,
           W1r, W1s, b1, W2, b2, Wg, bg, Wu1x, Wu1a, bu1, Wu2, bu2,
           _trace=False, _trace_kwargs=None):
    import concourse.mybir as mybir
    from concourse import bass_utils

    f4 = np.float32
    f8np = mybir.dt.np(mybir.dt.float8e4)
    x = np.asarray(node_features, f4)
    ph = np.asarray(node_phases, f4)
    adj = np.asarray(adjacency)

    mask = (adj != 0)
    counts = np.maximum(mask.sum(axis=1), 1).astype(f4)           # (N,)
    cinv_full = (1.0 / counts)                                     # (N,)

    xt_full = np.ascontiguousarray(x.transpose(0, 2, 1))           # (B, D, N)
    p8_full = np.ascontiguousarray(
        np.concatenate([np.cos(ph), np.sin(ph)], axis=2).transpose(0, 2, 1)
    )                                                              # (B, 8, N)
    wg8 = np.concatenate([np.asarray(Wg, f4), np.asarray(Wg, f4)], axis=0)

    # P8 replicated into the four 32-partition row groups
    p8q_full = np.zeros((B, D, N), f4)
    for g in range(4):
        p8q_full[:, 32 * g:32 * g + 2 * O, :] = p8_full
    # Wg8 stacked identically
    wg8stack = np.zeros((D, D), f4)
    for g in range(4):
        wg8stack[32 * g:32 * g + 2 * O, :] = wg8

    common = dict(xt=xt_full, p8q=p8q_full,
                  ones1=np.ones((1, D), f8np))

    cidx = np.arange(NCH)
    in_maps = []
    for core in range(NCORES):
        lo, hi = core * NPC, (core + 1) * NPC
        m = dict(common)
        m["xtr"] = np.ascontiguousarray(
            np.concatenate([xt_full[b][:, lo:hi] for b in range(B)], axis=1))
        p8r_core = p8_full[:, :, lo:hi]                            # (B, 8, NPC)
        p8rc = np.zeros((B, D, NCH), f4)
        for g in range(4):
            p8rc[:, 32 * g:32 * g + 2 * O, :] = p8r_core[:, :, cidx * 4 + g]
        m["p8rc"] = np.ascontiguousarray(p8rc)
        m["mneg8"] = np.ascontiguousarray(
            (MASK_NEG * (~mask[lo:hi])).astype(f8np).reshape(1, NPC * N))
        cinvb = np.broadcast_to(cinv_full[lo:hi][None, :], (D, NPC))
        m["blob"] = np.ascontiguousarray(np.concatenate(
            [np.asarray(W1r, f4), np.asarray(W1s, f4), np.asarray(W2, f4),
             np.asarray(Wu1x, f4), np.asarray(Wu1a, f4), np.asarray(Wu2, f4),
             wg8stack, cinvb, cinvb,
             np.asarray(b1, f4).reshape(D, 1), np.asarray(bg, f4).reshape(D, 1),
             np.asarray(b2, f4).reshape(D, 1), np.asarray(bu1, f4).reshape(D, 1),
             np.asarray(bu2, f4).reshape(D, 1)], axis=1))
        in_maps.append(m)

    nc = _get_program()
    if _trace:
        res = bass_utils.run_bass_kernel_spmd(
            nc, in_maps, list(range(NCORES)),
            trace=_trace, **(_trace_kwargs or {}))
        results = res.results
        kernel.last_results = res
    else:
        results = _get_runner(nc)(in_maps)

    out = np.empty((B, N, D), f4)
    for core in range(NCORES):
        lo, hi = core * NPC, (core + 1) * NPC
        out[:, lo:hi, :] = results[core]["out"].transpose(0, 2, 1)

    return out
